# revision 10
# baseline (speedup 1.0000x reference)
"""GATv2 message-passing kernel for 8 Trainium2 NeuronCores (Bass/Tile).

Strategy (per sharding hint): edges sharded contiguously across 8 cores,
node features replicated. Each core:
  1. projects all nodes -> DRAM table [NPAD, 64] (PE matmuls, lhsT = host-
     transposed node features so the contraction dim sits on partitions)
  2. streams its edge shard in chunks: 128-row indirect-DMA gathers of
     send/recv projections, PE edge-feature projection, ACT Mish, DVE
     logits + exp (max-free softmax: logits are bounded ~|10| for this
     model, exp stays in f32 range)
  3. scatter-adds per-edge payload [w*send(64); w(8)] into NT accumulator
     tables via CCE-add indirect DMAs. Intra-128 duplicate receivers are
     pre-summed with a PE selection-matrix matmul; only the first
     occurrence keeps the real row index (dups -> trash row), so every
     scatter DMA has unique indices; cross-DMA ordering per table is
     serialized by Tile's WAW tracking.
  4. merges tables, ReduceScatter over the 8 cores, divides numerator by
     denominator, writes its node-range output shard.
Host assembles the 8 shards.
"""
import sys
import os

_HERE = os.path.dirname(os.path.abspath(__file__))
sys.path.insert(0, _HERE)
import bass_compat  # noqa: F401  (walrus sem-wait limit workaround)

sys.path.insert(0, "/opt/trn_rl_repo")
import numpy as np
import concourse.bass as bass
import concourse.mybir as mybir
import concourse.tile as tile
from concourse.masks import make_identity

F32 = mybir.dt.float32
I32 = mybir.dt.int32

N_NODES = 50000
N_EDGES = 1200000
IN_DIM = 128
EDGE_DIM = 64
EMBED = 64
HEADS = 8
PAY = EMBED + HEADS  # 72

N_CORES = 8
EPC = N_EDGES // N_CORES  # 150000
CHUNK = 2048
NCH = (EPC + CHUNK - 1) // CHUNK  # 74
NPAD = 50176  # 392*128
TRASH = 50100
PADNODE = 50150
NT = 8  # accumulator tables
NQ = 1  # SWDGE queues (multi-queue payload rejected by this walrus path)
RSROWS = NPAD // N_CORES  # 6272
RSC = RSROWS // 128  # 49


def _ap3(ap, mid_n):
    """[128, D] AP -> [128, mid_n(step0), D] broadcast view."""
    return bass.AP(ap.tensor, ap.offset, [ap.ap[0], [0, mid_n]] + list(ap.ap[1:]))


def _inner_b(ap, n):
    """Append a step-0 innermost free dim of size n (broadcast view)."""
    return bass.AP(ap.tensor, ap.offset, list(ap.ap) + [[0, n]])


def _q(bi, q):
    if q:
        bi.ins.queue = f"qPoolDynamic{q}"
    return bi


def build_nc(nch=NCH):
    slots = nch * CHUNK
    cols = slots // 128
    nc = bass.Bass(num_swdge_queues=NQ)

    nfT = nc.declare_dram_parameter("nfT", [IN_DIM, NPAD], F32, isOutput=False)
    eftT = nc.declare_dram_parameter("eftT", [EDGE_DIM, slots], F32, isOutput=False)
    s_wrap = nc.declare_dram_parameter("s_wrap", [128, cols], I32, isOutput=False)
    r_wrap = nc.declare_dram_parameter("r_wrap", [128, cols], I32, isOutput=False)
    W_e = nc.declare_dram_parameter("W", [IN_DIM, EMBED], F32, isOutput=False)
    Wb_e = nc.declare_dram_parameter("Wb", [128, EMBED], F32, isOutput=False)
    We_e = nc.declare_dram_parameter("We", [EDGE_DIM, EMBED], F32, isOutput=False)
    Web_e = nc.declare_dram_parameter("Web", [128, EMBED], F32, isOutput=False)
    a_e = nc.declare_dram_parameter("a16", [128, 16 * EMBED], F32, isOutput=False)
    lmask_e = nc.declare_dram_parameter("lmask", [128, 128], F32, isOutput=False)
    out_e = nc.declare_dram_parameter("out_shard", [128, RSC, EMBED], F32, isOutput=True)

    table = nc.dram_tensor("ntable", [NPAD, EMBED], F32)
    accs = [nc.dram_tensor(f"acc{t}", [NPAD, PAY], F32) for t in range(NT)]
    merged = nc.dram_tensor("merged", [NPAD, PAY], F32)
    rs_out = nc.dram_tensor("rs_out", [RSROWS, PAY], F32)

    with tile.TileContext(nc) as tc:
        with (
            tc.tile_pool(name="const", bufs=1) as cpool,
            tc.tile_pool(name="nproj", bufs=3) as npool,
            tc.tile_pool(name="mrg", bufs=2) as mpool,
            tc.tile_pool(name="fine", bufs=1) as fpool,
            tc.tile_pool(name="edge", bufs=2) as epool,
            tc.tile_pool(name="small", bufs=2) as spool,
            tc.tile_pool(name="zeros", bufs=1) as zpool,
            tc.tile_pool(name="ps_e", bufs=1, space="PSUM") as ps_e,
            tc.tile_pool(name="ps_t", bufs=1, space="PSUM") as ps_t,
            tc.tile_pool(name="ps_r", bufs=1, space="PSUM") as ps_r,
            tc.tile_pool(name="ps_p", bufs=1, space="PSUM") as ps_p,
        ):
            # ---- constants
            W_t = cpool.tile([IN_DIM, EMBED], F32)
            nc.sync.dma_start(out=W_t[:], in_=W_e[:])
            Wb_t = cpool.tile([128, EMBED], F32)
            nc.sync.dma_start(out=Wb_t[:], in_=Wb_e[:])
            We_t = cpool.tile([EDGE_DIM, EMBED], F32)
            nc.sync.dma_start(out=We_t[:], in_=We_e[:])
            Web_t = cpool.tile([128, EMBED], F32)
            nc.sync.dma_start(out=Web_t[:], in_=Web_e[:])
            a_t = cpool.tile([128, 16 * EMBED], F32)
            nc.sync.dma_start(out=a_t[:], in_=a_e[:])
            lm_t = cpool.tile([128, 128], F32)
            nc.sync.dma_start(out=lm_t[:], in_=lmask_e[:])
            idt = cpool.tile([128, 128], F32)
            make_identity(nc, idt[:])
            ones_t = cpool.tile([128, 1], F32)
            nc.gpsimd.memset(ones_t[:], 1.0)
            zero1_t = cpool.tile([128, 16], F32)
            nc.gpsimd.memset(zero1_t[:], 0.0)
            trash_t = cpool.tile([128, 16], I32)
            nc.gpsimd.memset(trash_t[:], TRASH)

            # ---- zero the accumulator tables (big dense writes)
            zt = zpool.tile([128, 4096], F32)
            nc.gpsimd.memset(zt[:], 0.0)
            zflat_cols = NPAD * PAY // 128  # 28224
            for t in range(NT):
                flat = accs[t][:].rearrange("n d -> (n d)").rearrange(
                    "(p c) -> p c", p=128)
                c0 = 0
                while c0 < zflat_cols:
                    cw = min(4096, zflat_cols - c0)
                    nc.sync.dma_start(out=flat[:, c0:c0 + cw], in_=zt[:, :cw])
                    c0 += cw

            # ---- phase 1: node projection -> table
            for t in range(NPAD // 128):
                nf_t = npool.tile([IN_DIM, 128], F32, tag="nf")
                nc.sync.dma_start(out=nf_t[:], in_=nfT[:, t * 128:(t + 1) * 128])
                ps = ps_e.tile([128, 16, EMBED], F32, space="PSUM", tag="ep")
                nc.tensor.matmul(out=ps[:, 0, :], lhsT=nf_t[:], rhs=W_t[:],
                                 start=True, stop=True)
                nb = npool.tile([128, EMBED], F32, tag="nb")
                nc.vector.tensor_add(nb[:], ps[:, 0, :], Wb_t[:])
                nc.sync.dma_start(out=table[t * 128:(t + 1) * 128, :], in_=nb[:])

            # ---- phase 2: edge chunks
            for ch in range(nch):
                col0 = ch * 16
                s_t = epool.tile([128, 16], I32, tag="sidx")
                nc.sync.dma_start(out=s_t[:], in_=s_wrap[:, col0:col0 + 16])
                r_t = epool.tile([128, 16], I32, tag="ridx")
                nc.sync.dma_start(out=r_t[:], in_=r_wrap[:, col0:col0 + 16])
                ef_t = epool.tile([EDGE_DIM, CHUNK], F32, tag="eft")
                nc.sync.dma_start(
                    out=ef_t[:], in_=eftT[:, ch * CHUNK:(ch + 1) * CHUNK])

                gs = epool.tile([128, 16, EMBED], F32, tag="gs")
                gr = epool.tile([128, 16, EMBED], F32, tag="gr")
                for c in range(16):
                    _q(nc.gpsimd.indirect_dma_start(
                        out=gs[:, c, :], out_offset=None, in_=table[:],
                        in_offset=bass.IndirectOffsetOnAxis(ap=s_t[:, c:c + 1], axis=0),
                    ), (2 * c) % NQ)
                    _q(nc.gpsimd.indirect_dma_start(
                        out=gr[:, c, :], out_offset=None, in_=table[:],
                        in_offset=bass.IndirectOffsetOnAxis(ap=r_t[:, c:c + 1], axis=0),
                    ), (2 * c + 1) % NQ)

                pse = ps_e.tile([128, 16, EMBED], F32, space="PSUM", tag="ep")
                for c in range(16):
                    nc.tensor.matmul(
                        out=pse[:, c, :], lhsT=ef_t[:, c * 128:(c + 1) * 128],
                        rhs=We_t[:], start=True, stop=True)
                # x = gs + gr + eproj + Web
                x_t = epool.tile([128, 16, EMBED], F32, tag="x")
                xf = x_t[:].rearrange("p c d -> p (c d)")
                nc.vector.tensor_add(
                    xf, gs[:].rearrange("p c d -> p (c d)"),
                    gr[:].rearrange("p c d -> p (c d)"))
                nc.vector.tensor_add(xf, xf, pse[:].rearrange("p c d -> p (c d)"))
                nc.vector.tensor_add(x_t[:], x_t[:], _ap3(Web_t[:], 16))
                # mish(x) = x * tanh(softplus(x)) = x * (u^2+2u)/(u^2+2u+2), u=e^x
                # (exact; avoids an ACT table switch — only Exp is used)
                u_t = epool.tile([128, 16 * EMBED], F32, tag="mu")
                tb_t = epool.tile([128, 16 * EMBED], F32, tag="mtb")
                nc.scalar.activation(u_t[:], xf, mybir.ActivationFunctionType.Exp)
                nc.vector.tensor_scalar_add(tb_t[:], u_t[:], 2.0)
                nc.vector.tensor_mul(u_t[:], u_t[:], tb_t[:])  # A = u^2+2u
                nc.vector.tensor_scalar_add(tb_t[:], u_t[:], 2.0)  # B = A+2
                nc.vector.reciprocal(tb_t[:], tb_t[:])
                nc.vector.tensor_mul(u_t[:], u_t[:], tb_t[:])  # t = A/B
                xm_t = epool.tile([128, 16, EMBED], F32, tag="xm")
                nc.vector.tensor_mul(
                    xm_t[:].rearrange("p c d -> p (c d)"), xf, u_t[:])
                # logits: sum over inner 8 of xm * a
                lg_t = epool.tile([128, 16 * EMBED], F32, tag="lgm")
                nc.vector.tensor_mul(
                    lg_t[:], xm_t[:].rearrange("p c d -> p (c d)"), a_t[:])
                l_t = epool.tile([128, 16 * HEADS], F32, tag="lg")
                nc.vector.tensor_reduce(
                    l_t[:].rearrange("p (g o) -> p g o", o=1),
                    lg_t[:].rearrange("p (g i) -> p g i", i=8),
                    axis=mybir.AxisListType.X, op=mybir.AluOpType.add)
                w_t = epool.tile([128, 16 * HEADS], F32, tag="w")
                nc.scalar.activation(
                    w_t[:], l_t[:], mybir.ActivationFunctionType.Exp)
                # payload [128, 16, 72]: [:, :, :64] = gs * w (head-bcast), [:, 64:] = w
                pay_t = epool.tile([128, 16, PAY], F32, tag="pay")
                nc.vector.tensor_mul(
                    pay_t[:, :, :EMBED].rearrange("p c (h o) -> p c h o", o=8),
                    gs[:].rearrange("p c (h o) -> p c h o", o=8),
                    _inner_b(w_t[:].rearrange("p (c h) -> p c h", h=8), 8))
                nc.vector.tensor_copy(
                    pay_t[:, :, EMBED:], w_t[:].rearrange("p (c h) -> p c h", h=8))

                # dedup + scatter, in 2 half-batches of 8 cols
                rf_t = spool.tile([128, 16], F32, tag="rf")
                nc.vector.tensor_copy(rf_t[:], r_t[:])
                for hb in range(2):
                    cset = list(range(hb * 8, hb * 8 + 8))
                    pst = ps_t.tile([128, 8, 128], F32, space="PSUM", tag="tr")
                    for j, c in enumerate(cset):
                        nc.tensor.transpose(
                            out=pst[:, j, :],
                            in_=rf_t[:, c:c + 1].to_broadcast([128, 128]),
                            identity=idt[:])
                    tsame = spool.tile([128, 8, 128], F32, tag="tsame")
                    nc.vector.tensor_tensor(
                        out=tsame[:],
                        in0=_inner_b(rf_t[:, hb * 8:hb * 8 + 8], 128),
                        in1=pst[:],
                        op=mybir.AluOpType.is_equal)
                    cmask = spool.tile([128, 8, 128], F32, tag="cmask")
                    nc.vector.tensor_mul(cmask[:], tsame[:], _ap3(lm_t[:], 8))
                    psr = ps_r.tile([128, 8], F32, space="PSUM", tag="rk")
                    psp = ps_p.tile([128, 8, 128], F32, space="PSUM", tag="pr")
                    for j, c in enumerate(cset):
                        nc.tensor.matmul(out=psr[:, j:j + 1], lhsT=cmask[:, j, :],
                                         rhs=ones_t[:], start=True, stop=True)
                        nc.tensor.matmul(out=psp[:, j, :PAY], lhsT=tsame[:, j, :],
                                         rhs=pay_t[:, c, :], start=True, stop=True)
                    occ = spool.tile([128, 8], F32, tag="occ")
                    nc.vector.tensor_tensor(out=occ[:], in0=psr[:],
                                            in1=zero1_t[:, :8],
                                            op=mybir.AluOpType.is_equal)
                    # sidx = occ ? recv : TRASH  (f32 arithmetic, exact < 2^24)
                    self_f = spool.tile([128, 8], F32, tag="self")
                    nc.vector.tensor_scalar_add(
                        self_f[:], rf_t[:, hb * 8:hb * 8 + 8], float(-TRASH))
                    nc.vector.tensor_mul(self_f[:], self_f[:], occ[:])
                    nc.vector.tensor_scalar_add(self_f[:], self_f[:], float(TRASH))
                    sidx = spool.tile([128, 8], I32, tag="scix")
                    nc.vector.tensor_copy(sidx[:], self_f[:])
                    pres = spool.tile([128, 8, PAY], F32, tag="pres")
                    nc.vector.tensor_copy(pres[:], psp[:, :, :PAY])
                    for j, c in enumerate(cset):
                        gcol = ch * 16 + c
                        _q(nc.gpsimd.indirect_dma_start(
                            out=accs[gcol % NT][:],
                            out_offset=bass.IndirectOffsetOnAxis(
                                ap=sidx[:, j:j + 1], axis=0),
                            in_=pres[:, j, :],
                            in_offset=None,
                            compute_op=mybir.AluOpType.add,
                        ), gcol % NQ)

            # ---- phase 3: merge NT tables -> merged
            MR = 28  # rows per partition per group; 392/28 = 14 groups
            for g in range(NPAD // (128 * MR)):
                r0 = g * 128 * MR
                mt = mpool.tile([128, MR, PAY], F32, tag="mg")
                nc.sync.dma_start(
                    out=mt[:],
                    in_=accs[0][r0:r0 + 128 * MR, :].rearrange(
                        "(p c) d -> p c d", p=128))
                for t in range(1, NT):
                    at = mpool.tile([128, MR, PAY], F32, tag="ma")
                    nc.sync.dma_start(
                        out=at[:],
                        in_=accs[t][r0:r0 + 128 * MR, :].rearrange(
                            "(p c) d -> p c d", p=128))
                    nc.vector.tensor_add(
                        mt[:].rearrange("p c d -> p (c d)"),
                        mt[:].rearrange("p c d -> p (c d)"),
                        at[:].rearrange("p c d -> p (c d)"))
                nc.sync.dma_start(
                    out=merged[r0:r0 + 128 * MR, :].rearrange(
                        "(p c) d -> p c d", p=128),
                    in_=mt[:])

            # ---- phase 4: ReduceScatter + divide + out
            nc.gpsimd.collective_compute(
                "ReduceScatter",
                mybir.AluOpType.add,
                replica_groups=[list(range(N_CORES))],
                ins=[merged[:]],
                outs=[rs_out[:]])
            fin = fpool.tile([128, RSC, PAY], F32, tag="fin")
            nc.sync.dma_start(
                out=fin[:], in_=rs_out[:].rearrange("(p c) d -> p c d", p=128))
            den = fpool.tile([128, RSC, HEADS], F32, tag="den")
            nc.vector.tensor_scalar_add(den[:], fin[:, :, EMBED:], 1e-30)
            rec = fpool.tile([128, RSC, HEADS], F32, tag="rec")
            nc.vector.reciprocal(rec[:], den[:])
            ot = fpool.tile([128, RSC, EMBED], F32, tag="ot")
            nc.vector.tensor_mul(
                ot[:].rearrange("p c (h o) -> p c h o", o=8),
                fin[:, :, :EMBED].rearrange("p c (h o) -> p c h o", o=8),
                _inner_b(rec[:], 8))
            nc.sync.dma_start(out=out_e[:], in_=ot[:])

    bass_compat.split_excess_waits(nc)
    return nc


def host_prep(node_features, edge_features, senders, receivers,
              W_kernel, W_bias, We_kernel, We_bias, a,
              n_cores=N_CORES, nch=NCH):
    """Pure layout transforms -> per-core input maps."""
    slots = nch * CHUNK
    cols = slots // 128
    epc = min(EPC, slots)

    nf_pad = np.zeros((NPAD, IN_DIM), np.float32)
    nf_pad[:N_NODES] = node_features
    nfT = np.ascontiguousarray(nf_pad.T)

    Wb_rep = np.tile(np.asarray(W_bias, np.float32)[None, :], (128, 1))
    Web_rep = np.tile(np.asarray(We_bias, np.float32)[None, :], (128, 1))
    a16 = np.tile(np.asarray(a, np.float32).reshape(-1)[None, :], (128, 16))
    lmask = np.triu(np.ones((128, 128), np.float32), 1)

    in_maps = []
    for c in range(n_cores):
        lo = c * epc
        hi = min(lo + epc, len(senders))
        n_real = hi - lo
        s_pad = np.full(slots, PADNODE, np.int32)
        r_pad = np.full(slots, PADNODE, np.int32)
        ef_pad = np.zeros((slots, EDGE_DIM), np.float32)
        s_pad[:n_real] = senders[lo:hi]
        r_pad[:n_real] = receivers[lo:hi]
        ef_pad[:n_real] = edge_features[lo:hi]
        in_maps.append({
            "nfT": nfT,
            "eftT": np.ascontiguousarray(ef_pad.T),
            "s_wrap": np.ascontiguousarray(s_pad.reshape(cols, 128).T),
            "r_wrap": np.ascontiguousarray(r_pad.reshape(cols, 128).T),
            "W": np.asarray(W_kernel, np.float32),
            "Wb": Wb_rep,
            "We": np.asarray(We_kernel, np.float32),
            "Web": Web_rep,
            "a16": a16,
            "lmask": lmask,
        })
    return in_maps


_CACHE = {}


def kernel(node_features, edge_features, global_features, senders, receivers,
           W_kernel, W_bias, We_kernel, We_bias, a):
    node_features = np.asarray(node_features, np.float32)
    edge_features = np.asarray(edge_features, np.float32)
    senders = np.asarray(senders, np.int32)
    receivers = np.asarray(receivers, np.int32)
    in_maps = host_prep(node_features, edge_features, senders, receivers,
                        W_kernel, W_bias, We_kernel, We_bias, a)
    if "fn" not in _CACHE:
        import bench_utils
        nc = build_nc()
        fn, _, _ = bench_utils.build_runner(nc, N_CORES)
        _CACHE["fn"] = fn
    res, dt = _CACHE["fn"](in_maps)
    _CACHE["last_dt"] = dt
    # out_shard [128, RSC, 64]; row index within shard = p*RSC + c
    full = np.concatenate(
        [r["out_shard"].reshape(128 * RSC, EMBED) for r in res], axis=0)
    return full[:N_NODES].astype(np.float32)
